# revision 5
# baseline (speedup 1.0000x reference)
"""Trainium2 Bass kernel for nn_SamplePolicy_14886356648064.

Reference semantics (T=4 resample rounds, K=4 vote threshold, H=8 heads):
  each round: per-head argmax over src -> presence vector per head ->
  counting = sum of presence over heads -> trigger = counting.max() <= K ->
  if trigger, replace all heads with head `sampled_t` (broadcast).

Exact algebraic collapse of the T-loop:
  - If trigger is False for the initial aw, the state never changes, so the
    trigger value is identical every round -> output == input.
  - If trigger fires (round 0), all heads become identical; afterwards each
    src position in the (common) argmax set receives H=8 votes > K=4, so the
    trigger can never fire again -> output == broadcast(aw[sampled_0]).
  Hence only round 0's trigger and sampled_0 matter, for ANY input.

Device work: the input is cast on the host to fp16 (a monotone, pointwise
rounding -- halves HBM bytes), sharded one head per NeuronCore.  Each core
streams its 16.8MB fp16 head slice once and reduces each row window to 256
"cell" maxima, where cell c of a window [c0, c0+W) holds positions
{c0 + c + 256*k}.  The reduction is an in-place contiguous halving tree on
the vector engine (tensor_tensor max runs at 2 elem/cycle/lane for 16-bit
dtypes, vs 1x for tensor_reduce), with fold ops batched over pairs of row
tiles to amortize the ~200ns/op instruction overhead.  The first two tiles
load and fold as 2048-col halves so the fold train starts as early as
possible; cell maxima flush to DRAM in two chunks over the otherwise-idle
SWDGE queue so the store overlaps the load stream.  The host then gathers
only the candidate cells (those attaining the row's fp16 max -- a monotone
map guarantees the true argmax's cell is among them), resolves the exact
first-occurrence f32 argmax, and runs the tiny vote/trigger logic.

sampled_0 = jax.random.randint(jax.random.fold_in(jax.random.key(42), 0),
                               (), 0, 7) == 3 (threefry, platform independent).
"""

import numpy as np

H = 8
TGT = 2048
SRC = 4096
P = 128            # SBUF partitions per tile
NTILES = TGT // P  # 16 row tiles per head
C = 256            # cells per chunk (residues mod 256 of the chunk window)
K_THRESH = 4
SAMPLED_T0 = 3

# chunk slots, in load/fold order: (tile, col0, width)
CHUNKS = (
    [(0, 0, 2048), (0, 2048, 2048), (1, 0, 2048), (1, 2048, 2048)]
    + [(t, 0, SRC) for t in range(2, 15)]
    + [(15, 0, 2048), (15, 2048, 2048)]
)
NCHUNK = len(CHUNKS)  # 19
N_STORE_A = 12        # chunk slots flushed early (t0..t9)

# load/fold units: (slot0, nslots) — a unit is one DMA + one fold train.
# Pairs of full tiles share fold ops (halved per-op overhead); the early
# half-tile units start the fold train ~10us into the kernel.
UNITS = (
    [(0, 1), (1, 1), (2, 1), (3, 1), (4, 1), (5, 1)]           # t0a..t3
    + [(6, 2), (8, 2), (10, 2), (12, 2), (14, 2)]              # t4..t13 pairs
    + [(16, 1), (17, 1), (18, 1)]                              # t14, t15a/b
)

_cache = {}


def _build_nc():
    """Raw Bass program, one head per core, fp16 input.

    All 16 row-tiles are SBUF-resident (128KB/partition).  Loads alternate
    between the two HWDGE rings; the vector engine runs an in-place halving
    max-tree per unit as soon as that unit's load lands.  Cells of a window
    end up in the window's first 256 columns.
    """
    from contextlib import ExitStack

    import concourse.bass as bass
    import concourse.mybir as mybir

    nc = bass.Bass()
    f16 = mybir.dt.float16
    x = nc.declare_dram_parameter("x", [TGT, SRC], f16, isOutput=False)
    bm = nc.declare_dram_parameter("bm", [P, NCHUNK, C], f16, isOutput=True)

    with ExitStack() as ctx:
        tiles = ctx.enter_context(nc.sbuf_tensor([P, NTILES, SRC], f16))
        bmsb = ctx.enter_context(nc.sbuf_tensor([P, NCHUNK, C], f16))
        s_u = [ctx.enter_context(nc.semaphore(f"s_u{j}")) for j in range(len(UNITS))]
        s_da = ctx.enter_context(nc.semaphore("s_da"))
        s_db = ctx.enter_context(nc.semaphore("s_db"))
        s_out = ctx.enter_context(nc.semaphore("s_out"))
        block = ctx.enter_context(nc.Block())

        # DRAM view [p, tile, col] of the row-major [2048, 4096] head slice
        xv = x[:, :].rearrange("(t p) c -> p t c", p=P)

        def unit_src(slot0, nslots):
            """(tile0, ntiles, col0, width) covered by a unit."""
            t0, c0, w = CHUNKS[slot0]
            if nslots == 2:
                t1, c1, w1 = CHUNKS[slot0 + 1]
                assert c0 == c1 == 0 and w == w1 == SRC and t1 == t0 + 1
                return t0, 2, 0, SRC
            return t0, 1, c0, w

        def issue_loads(eng, parity):
            for j, (slot0, nslots) in enumerate(UNITS):
                if j % 2 != parity:
                    continue
                t0, nt, c0, w = unit_src(slot0, nslots)
                eng.dma_start(
                    out=tiles[:, t0 : t0 + nt, c0 : c0 + w],
                    in_=xv[:, t0 : t0 + nt, c0 : c0 + w],
                ).then_inc(s_u[j], 16)

        @block.sync
        def _(sync):
            issue_loads(sync, 0)
            sync.wait_ge(s_out, 32)

        @block.scalar
        def _(scalar):
            issue_loads(scalar, 1)

        @block.gpsimd
        def _(gpsimd):
            # bm stores ride the (otherwise idle) SWDGE queue so the early
            # flush overlaps the HWDGE load stream and tail folds.
            gpsimd.wait_ge(s_da, 1)
            gpsimd.dma_start(
                out=bm[:, :N_STORE_A, :], in_=bmsb[:, :N_STORE_A, :]
            ).then_inc(s_out, 16)
            gpsimd.wait_ge(s_db, 1)
            gpsimd.dma_start(
                out=bm[:, N_STORE_A:, :], in_=bmsb[:, N_STORE_A:, :]
            ).then_inc(s_out, 16)

        @block.vector
        def _(vector):
            mx = mybir.AluOpType.max

            for j, (slot0, nslots) in enumerate(UNITS):
                vector.wait_ge(s_u[j], 16)
                t0, nt, c0, w0 = unit_src(slot0, nslots)
                w = w0 // 2
                while w >= C:
                    nc.vector.tensor_tensor(
                        out=tiles[:, t0 : t0 + nt, c0 : c0 + w],
                        in0=tiles[:, t0 : t0 + nt, c0 : c0 + w],
                        in1=tiles[:, t0 : t0 + nt, c0 + w : c0 + 2 * w],
                        op=mx,
                    )
                    w //= 2
                if t0 + nt == 10 and c0 == 0 and nt == 2:
                    # tiles 0..9 folded: flush chunk slots 0..11.
                    # slots 0-3 = tile 0/1 half windows, slots 4-11 = t2..t9.
                    nc.vector.tensor_copy(
                        out=bmsb[:, 0:4, :].rearrange(
                            "p (t w) c -> p t w c", t=2
                        ),
                        in_=tiles[:, 0:2, :].rearrange(
                            "p t (w c) -> p t w c", w=2
                        )[:, :, :, 0:C],
                    )
                    nc.vector.tensor_copy(
                        out=bmsb[:, 4:N_STORE_A, :], in_=tiles[:, 2:10, 0:C]
                    ).then_inc(s_da, 1)
            # remaining cells: slots 12..16 = t10..t14, slots 17/18 = tile 15
            # half windows.
            nc.vector.tensor_copy(
                out=bmsb[:, N_STORE_A : N_STORE_A + 5, :],
                in_=tiles[:, 10:15, 0:C],
            )
            nc.vector.tensor_copy(
                out=bmsb[:, N_STORE_A + 5 :, :].rearrange(
                    "p (t w) c -> p t w c", t=1
                ),
                in_=tiles[:, 15:16, :].rearrange("p t (w c) -> p t w c", w=2)[
                    :, :, :, 0:C
                ],
            ).then_inc(s_db, 1)

    return nc


def _get_nc():
    if "nc" not in _cache:
        _cache["nc"] = _build_nc()
    return _cache["nc"]


def run_device(aw16, **run_kwargs):
    """Run the per-head cell-max kernel on 8 cores.

    aw16: [H, TGT, SRC] float16. Returns ([H, P, NCHUNK, C] float16, results).
    """
    from concourse.bass_utils import run_bass_kernel_spmd

    nc = _get_nc()
    in_maps = [{"x": np.ascontiguousarray(aw16[c])} for c in range(H)]
    res = run_bass_kernel_spmd(nc, in_maps, list(range(H)), **run_kwargs)
    bm = np.stack([res.results[c]["bm"] for c in range(H)])
    return bm, res


def _host_cellmax(aw16):
    """Numpy fallback producing the same [H, P, NCHUNK, C] cell maxima."""
    bm = np.empty((H, P, NCHUNK, C), np.float16)
    for s, (t, c0, w) in enumerate(CHUNKS):
        rows = aw16[:, t * P : (t + 1) * P, c0 : c0 + w]
        bm[:, :, s, :] = rows.reshape(H, P, w // C, C).max(axis=2)
    return bm


def _exact_argmax(aw, bm):
    """Exact first-occurrence np.argmax(aw, -1) from device cell maxima.

    aw: [H, TGT, SRC] float32; bm: [H, P, NCHUNK, C] float16 with the CHUNKS
    layout: chunk slot s covers rows t*128+p of column window [c0, c0+w),
    cell c holding positions {c0 + c + 256k : k < w//256}.
    """
    BIG = 1 << 20
    cand_m, cand_pos, cand_row = [], [], []

    # per-row fp16 max over all cells of that row's chunks
    rowmax = np.full((H, TGT), -np.inf, np.float16)
    for s, (t, c0, w) in enumerate(CHUNKS):
        np.maximum(
            rowmax[:, t * P : (t + 1) * P],
            bm[:, :, s, :].max(-1),
            out=rowmax[:, t * P : (t + 1) * P],
        )

    for s, (t, c0, w) in enumerate(CHUNKS):
        depth = w // C
        mask = bm[:, :, s, :] == rowmax[:, t * P : (t + 1) * P, None]
        hs, ps, cs = np.nonzero(mask)
        if not hs.size:
            continue
        win = aw[:, t * P : (t + 1) * P, c0 : c0 + w].reshape(H, P, depth, C)
        vals = win[hs, ps, :, cs]  # [N, depth]
        m = vals.max(1)
        k = np.where(vals == m[:, None], np.arange(depth), BIG).min(1)
        cand_m.append(m)
        cand_pos.append(c0 + cs + C * k)
        cand_row.append(hs * TGT + (t * P + ps))

    m = np.concatenate(cand_m)
    pos = np.concatenate(cand_pos)
    row = np.concatenate(cand_row)
    order = np.argsort(row, kind="stable")
    m, pos, row = m[order], pos[order], row[order]
    starts = np.flatnonzero(np.r_[True, row[1:] != row[:-1]])
    urow = row[starts]
    assert urow.size == H * TGT, f"missing rows: {urow.size}"
    best = np.maximum.reduceat(m, starts)
    seg = np.repeat(np.arange(starts.size), np.diff(np.r_[starts, row.size]))
    bestpos = np.minimum.reduceat(
        np.where(m == best[seg], pos, 1 << 30), starts
    )
    out = np.empty(H * TGT, np.int64)
    out[urow] = bestpos
    return out.reshape(H, TGT)


def kernel(attention_weight):
    aw = np.asarray(attention_weight)
    assert aw.shape == (H, TGT, SRC), aw.shape
    aw = aw.astype(np.float32, copy=False)
    aw16 = aw.astype(np.float16)

    try:
        bm, _ = run_device(aw16)
    except Exception as e:  # device path failed: fall back to host cellmax
        import traceback

        traceback.print_exc()
        print(f"WARNING: device path failed ({e!r}); falling back to numpy")
        bm = _host_cellmax(aw16)

    cand = _exact_argmax(aw, bm)  # [H, TGT]
    present = np.zeros((H, SRC), np.float32)
    present[np.arange(H)[:, None], cand] = 1.0
    counting = present.sum(axis=0)

    if counting.max() <= K_THRESH:
        return np.broadcast_to(aw[SAMPLED_T0], aw.shape).copy()
    return aw


# revision 10
# speedup vs baseline: 1.0122x; 1.0122x over previous
"""Trainium2 Bass kernel for nn_SamplePolicy_14886356648064.

Reference semantics (T=4 resample rounds, K=4 vote threshold, H=8 heads):
  each round: per-head argmax over src -> presence vector per head ->
  counting = sum of presence over heads -> trigger = counting.max() <= K ->
  if trigger, replace all heads with head `sampled_t` (broadcast).

Exact algebraic collapse of the T-loop:
  - If trigger is False for the initial aw, the state never changes, so the
    trigger value is identical every round -> output == input.
  - If trigger fires (round 0), all heads become identical; afterwards each
    src position in the (common) argmax set receives H=8 votes > K=4, so the
    trigger can never fire again -> output == broadcast(aw[sampled_0]).
  Hence only round 0's trigger and sampled_0 matter, for ANY input.

Device work: the input is cast on the host to fp16 (a monotone, pointwise
rounding -- halves HBM bytes), sharded one head per NeuronCore.  Each core
streams its 16.8MB fp16 head slice once and reduces each row window to 256
"cell" maxima, where cell c of a window [c0, c0+W) holds positions
{c0 + c + 256*k}.  The reduction is an in-place contiguous halving tree on
the vector engine (tensor_tensor max runs at 2 elem/cycle/lane for 16-bit
dtypes, vs 1x for tensor_reduce), with fold ops batched over pairs of row
tiles to amortize the ~200ns/op instruction overhead.  The first two tiles
load and fold as 2048-col halves so the fold train starts as early as
possible; cell maxima flush to DRAM in two chunks over the otherwise-idle
SWDGE queue so the store overlaps the load stream.  The host then gathers
only the candidate cells (those attaining the row's fp16 max -- a monotone
map guarantees the true argmax's cell is among them), resolves the exact
first-occurrence f32 argmax, and runs the tiny vote/trigger logic.

sampled_0 = jax.random.randint(jax.random.fold_in(jax.random.key(42), 0),
                               (), 0, 7) == 3 (threefry, platform independent).
"""

import numpy as np

H = 8
TGT = 2048
SRC = 4096
P = 128            # SBUF partitions per tile
NTILES = TGT // P  # 16 row tiles per head
C = 256            # cells per chunk (residues mod 256 of the chunk window)
K_THRESH = 4
SAMPLED_T0 = 3

# chunk slots, in load/fold order: (tile, col0, width)
CHUNKS = (
    [(0, 0, 2048), (0, 2048, 2048), (1, 0, 2048), (1, 2048, 2048)]
    + [(t, 0, SRC) for t in range(2, 15)]
    + [(15, 0, 2048), (15, 2048, 1024), (15, 3072, 1024)]
)
NCHUNK = len(CHUNKS)  # 20
N_STORE_A = 12        # chunk slots flushed early (t0..t9)
N_STORE_B = 17        # chunk slots of the second flush (t10..t14)

# load/fold units: (slot0, nslots) — a unit is one DMA + one fold train.
# Pairs of full tiles share fold ops (halved per-op overhead); the early
# half-tile units start the fold train ~10us into the kernel, and the last
# tile tapers into quarter-window units so the post-stream tail is short.
UNITS = (
    [(0, 1), (1, 1), (2, 1), (3, 1), (4, 1), (5, 1)]           # t0a..t3
    + [(6, 2), (8, 2), (10, 2), (12, 2), (14, 2)]              # t4..t13 pairs
    + [(16, 1), (17, 1), (18, 1), (19, 1)]                     # t14, t15 taper
)

_cache = {}


def _build_nc():
    """Raw Bass program, one head per core, fp16 input.

    All 16 row-tiles are SBUF-resident (128KB/partition).  Loads alternate
    between the two HWDGE rings; the vector engine runs an in-place halving
    max-tree per unit as soon as that unit's load lands.  Cells of a window
    end up in the window's first 256 columns.
    """
    from contextlib import ExitStack

    import concourse.bass as bass
    import concourse.mybir as mybir

    nc = bass.Bass()
    f16 = mybir.dt.float16
    x = nc.declare_dram_parameter("x", [TGT, SRC], f16, isOutput=False)
    bm = nc.declare_dram_parameter("bm", [P, NCHUNK, C], f16, isOutput=True)

    with ExitStack() as ctx:
        tiles = ctx.enter_context(nc.sbuf_tensor([P, NTILES, SRC], f16))
        bmsb = ctx.enter_context(nc.sbuf_tensor([P, NCHUNK, C], f16))
        s_u = [ctx.enter_context(nc.semaphore(f"s_u{j}")) for j in range(len(UNITS))]
        s_da = ctx.enter_context(nc.semaphore("s_da"))
        s_db = ctx.enter_context(nc.semaphore("s_db"))
        s_dc = ctx.enter_context(nc.semaphore("s_dc"))
        s_out = ctx.enter_context(nc.semaphore("s_out"))
        block = ctx.enter_context(nc.Block())

        # DRAM view [p, tile, col] of the row-major [2048, 4096] head slice
        xv = x[:, :].rearrange("(t p) c -> p t c", p=P)

        def unit_src(slot0, nslots):
            """(tile0, ntiles, col0, width) covered by a unit."""
            t0, c0, w = CHUNKS[slot0]
            if nslots == 2:
                t1, c1, w1 = CHUNKS[slot0 + 1]
                assert c0 == c1 == 0 and w == w1 == SRC and t1 == t0 + 1
                return t0, 2, 0, SRC
            return t0, 1, c0, w

        def issue_loads(eng, parity):
            for j, (slot0, nslots) in enumerate(UNITS):
                if j % 2 != parity:
                    continue
                t0, nt, c0, w = unit_src(slot0, nslots)
                eng.dma_start(
                    out=tiles[:, t0 : t0 + nt, c0 : c0 + w],
                    in_=xv[:, t0 : t0 + nt, c0 : c0 + w],
                ).then_inc(s_u[j], 16)

        @block.sync
        def _(sync):
            issue_loads(sync, 0)
            sync.wait_ge(s_out, 48)

        @block.scalar
        def _(scalar):
            issue_loads(scalar, 1)

        @block.gpsimd
        def _(gpsimd):
            # bm stores ride the (otherwise idle) SWDGE queue so the early
            # flushes overlap the HWDGE load stream and tail folds, leaving
            # only a tiny final store after the last fold.
            gpsimd.wait_ge(s_da, 1)
            gpsimd.dma_start(
                out=bm[:, :N_STORE_A, :], in_=bmsb[:, :N_STORE_A, :]
            ).then_inc(s_out, 16)
            gpsimd.wait_ge(s_db, 1)
            gpsimd.dma_start(
                out=bm[:, N_STORE_A:N_STORE_B, :],
                in_=bmsb[:, N_STORE_A:N_STORE_B, :],
            ).then_inc(s_out, 16)
            gpsimd.wait_ge(s_dc, 1)
            gpsimd.dma_start(
                out=bm[:, N_STORE_B:, :], in_=bmsb[:, N_STORE_B:, :]
            ).then_inc(s_out, 16)

        @block.vector
        def _(vector):
            mx = mybir.AluOpType.max

            for j, (slot0, nslots) in enumerate(UNITS):
                vector.wait_ge(s_u[j], 16)
                t0, nt, c0, w0 = unit_src(slot0, nslots)
                w = w0 // 2
                while w >= C:
                    nc.vector.tensor_tensor(
                        out=tiles[:, t0 : t0 + nt, c0 : c0 + w],
                        in0=tiles[:, t0 : t0 + nt, c0 : c0 + w],
                        in1=tiles[:, t0 : t0 + nt, c0 + w : c0 + 2 * w],
                        op=mx,
                    )
                    w //= 2
                if t0 + nt == 10 and c0 == 0 and nt == 2:
                    # tiles 0..9 folded: flush chunk slots 0..11.
                    # slots 0-3 = tile 0/1 half windows, slots 4-11 = t2..t9.
                    nc.vector.tensor_copy(
                        out=bmsb[:, 0:4, :].rearrange(
                            "p (t w) c -> p t w c", t=2
                        ),
                        in_=tiles[:, 0:2, :].rearrange(
                            "p t (w c) -> p t w c", w=2
                        )[:, :, :, 0:C],
                    )
                    nc.vector.tensor_copy(
                        out=bmsb[:, 4:N_STORE_A, :], in_=tiles[:, 2:10, 0:C]
                    ).then_inc(s_da, 1)
                if t0 == 14 and nt == 1 and c0 == 0:
                    # t14 folded: flush slots 12..16 (t10..t14)
                    nc.vector.tensor_copy(
                        out=bmsb[:, N_STORE_A:N_STORE_B, :],
                        in_=tiles[:, 10:15, 0:C],
                    ).then_inc(s_db, 1)
            # tile-15 cells: slot 17 = window [0,2048), slot 18 = [2048,3072),
            # slot 19 = [3072,4096).
            nc.vector.tensor_copy(
                out=bmsb[:, N_STORE_B, :], in_=tiles[:, 15, 0:C]
            )
            nc.vector.tensor_copy(
                out=bmsb[:, N_STORE_B + 1 :, :].rearrange(
                    "p (t w) c -> p t w c", t=1
                ),
                in_=tiles[:, 15:16, SRC // 2 :].rearrange(
                    "p t (w c) -> p t w c", w=2
                )[:, :, :, 0:C],
            ).then_inc(s_dc, 1)

    return nc


def _get_nc():
    if "nc" not in _cache:
        _cache["nc"] = _build_nc()
    return _cache["nc"]


def run_device(aw16, **run_kwargs):
    """Run the per-head cell-max kernel on 8 cores.

    aw16: [H, TGT, SRC] float16. Returns ([H, P, NCHUNK, C] float16, results).
    """
    from concourse.bass_utils import run_bass_kernel_spmd

    nc = _get_nc()
    in_maps = [{"x": np.ascontiguousarray(aw16[c])} for c in range(H)]
    res = run_bass_kernel_spmd(nc, in_maps, list(range(H)), **run_kwargs)
    bm = np.stack([res.results[c]["bm"] for c in range(H)])
    return bm, res


def _host_cellmax(aw16):
    """Numpy fallback producing the same [H, P, NCHUNK, C] cell maxima."""
    bm = np.empty((H, P, NCHUNK, C), np.float16)
    for s, (t, c0, w) in enumerate(CHUNKS):
        rows = aw16[:, t * P : (t + 1) * P, c0 : c0 + w]
        bm[:, :, s, :] = rows.reshape(H, P, w // C, C).max(axis=2)
    return bm


def _exact_argmax(aw, bm):
    """Exact first-occurrence np.argmax(aw, -1) from device cell maxima.

    aw: [H, TGT, SRC] float32; bm: [H, P, NCHUNK, C] float16 with the CHUNKS
    layout: chunk slot s covers rows t*128+p of column window [c0, c0+w),
    cell c holding positions {c0 + c + 256k : k < w//256}.
    """
    BIG = 1 << 20
    cand_m, cand_pos, cand_row = [], [], []

    # per-row fp16 max over all cells of that row's chunks
    rowmax = np.full((H, TGT), -np.inf, np.float16)
    for s, (t, c0, w) in enumerate(CHUNKS):
        np.maximum(
            rowmax[:, t * P : (t + 1) * P],
            bm[:, :, s, :].max(-1),
            out=rowmax[:, t * P : (t + 1) * P],
        )

    for s, (t, c0, w) in enumerate(CHUNKS):
        depth = w // C
        mask = bm[:, :, s, :] == rowmax[:, t * P : (t + 1) * P, None]
        hs, ps, cs = np.nonzero(mask)
        if not hs.size:
            continue
        win = aw[:, t * P : (t + 1) * P, c0 : c0 + w].reshape(H, P, depth, C)
        vals = win[hs, ps, :, cs]  # [N, depth]
        m = vals.max(1)
        k = np.where(vals == m[:, None], np.arange(depth), BIG).min(1)
        cand_m.append(m)
        cand_pos.append(c0 + cs + C * k)
        cand_row.append(hs * TGT + (t * P + ps))

    m = np.concatenate(cand_m)
    pos = np.concatenate(cand_pos)
    row = np.concatenate(cand_row)
    order = np.argsort(row, kind="stable")
    m, pos, row = m[order], pos[order], row[order]
    starts = np.flatnonzero(np.r_[True, row[1:] != row[:-1]])
    urow = row[starts]
    assert urow.size == H * TGT, f"missing rows: {urow.size}"
    best = np.maximum.reduceat(m, starts)
    seg = np.repeat(np.arange(starts.size), np.diff(np.r_[starts, row.size]))
    bestpos = np.minimum.reduceat(
        np.where(m == best[seg], pos, 1 << 30), starts
    )
    out = np.empty(H * TGT, np.int64)
    out[urow] = bestpos
    return out.reshape(H, TGT)


def kernel(attention_weight):
    aw = np.asarray(attention_weight)
    assert aw.shape == (H, TGT, SRC), aw.shape
    aw = aw.astype(np.float32, copy=False)
    aw16 = aw.astype(np.float16)

    try:
        bm, _ = run_device(aw16)
    except Exception as e:  # device path failed: fall back to host cellmax
        import traceback

        traceback.print_exc()
        print(f"WARNING: device path failed ({e!r}); falling back to numpy")
        bm = _host_cellmax(aw16)

    cand = _exact_argmax(aw, bm)  # [H, TGT]
    present = np.zeros((H, SRC), np.float32)
    present[np.arange(H)[:, None], cand] = 1.0
    counting = present.sum(axis=0)

    if counting.max() <= K_THRESH:
        return np.broadcast_to(aw[SAMPLED_T0], aw.shape).copy()
    return aw


# revision 12
# speedup vs baseline: 1.1146x; 1.1011x over previous
"""Trainium2 Bass kernel for nn_SamplePolicy_14886356648064.

Reference semantics (T=4 resample rounds, K=4 vote threshold, H=8 heads):
  each round: per-head argmax over src -> presence vector per head ->
  counting = sum of presence over heads -> trigger = counting.max() <= K ->
  if trigger, replace all heads with head `sampled_t` (broadcast).

Exact algebraic collapse of the T-loop:
  - If trigger is False for the initial aw, the state never changes, so the
    trigger value is identical every round -> output == input.
  - If trigger fires (round 0), all heads become identical; afterwards each
    src position in the (common) argmax set receives H=8 votes > K=4, so the
    trigger can never fire again -> output == broadcast(aw[sampled_0]).
  Hence only round 0's trigger and sampled_0 matter, for ANY input.

Device work: the input is cast on the host to fp16 (a monotone, pointwise
rounding -- halves HBM bytes), sharded one head per NeuronCore.  Each core
streams its 16.8MB fp16 head slice once and reduces each row window to 128
"cell" maxima, where cell c of a window [c0, c0+W) holds positions
{c0 + c + 128*k}.  The reduction is an in-place contiguous halving tree on
the vector engine (tensor_tensor max runs at 2 elem/cycle/lane for 16-bit
dtypes, vs 1x for tensor_reduce), with fold ops batched over pairs of row
tiles to amortize the ~200ns/op instruction overhead.  The first two tiles
load and fold as 2048-col halves so the fold train starts as early as
possible, and the last tile tapers into quarter windows so the post-stream
tail is short; cell maxima flush to DRAM in three chunks over the
otherwise-idle SWDGE queue so stores overlap the load stream.  The host
then gathers only the candidate cells (those attaining the row's fp16 max
-- a monotone map guarantees the true argmax's cell is among them),
resolves the exact first-occurrence f32 argmax, and runs the tiny
vote/trigger logic.

sampled_0 = jax.random.randint(jax.random.fold_in(jax.random.key(42), 0),
                               (), 0, 7) == 3 (threefry, platform independent).
"""

import numpy as np

H = 8
TGT = 2048
SRC = 4096
P = 128            # SBUF partitions per tile
NTILES = TGT // P  # 16 row tiles per head
C = 128            # cells per chunk (residues mod 128 of the chunk window)
K_THRESH = 4
SAMPLED_T0 = 3

# chunk slots, in load/fold order: (tile, col0, width)
CHUNKS = (
    [(0, 0, 2048), (0, 2048, 2048), (1, 0, 2048), (1, 2048, 2048)]
    + [(t, 0, SRC) for t in range(2, 15)]
    + [(15, 0, 2048), (15, 2048, 1024), (15, 3072, 1024)]
)
NCHUNK = len(CHUNKS)  # 20
N_STORE_A = 12        # chunk slots flushed early (t0..t9)
N_STORE_B = 17        # chunk slots of the second flush (t10..t14)

# load/fold units: (slot0, nslots) — a unit is one DMA + one fold train.
# Pairs of full tiles share fold ops (halved per-op overhead); the early
# half-tile units start the fold train ~10us into the kernel, and the last
# tile tapers into quarter-window units so the post-stream tail is short.
UNITS = (
    [(0, 1), (1, 1), (2, 1), (3, 1), (4, 1), (5, 1)]           # t0a..t3
    + [(6, 2), (8, 2), (10, 2), (12, 2), (14, 2)]              # t4..t13 pairs
    + [(16, 1), (17, 1), (18, 1), (19, 1)]                     # t14, t15 taper
)

_cache = {}


def _build_nc():
    """Raw Bass program, one head per core, fp16 input.

    All 16 row-tiles are SBUF-resident (128KB/partition).  Loads alternate
    between the two HWDGE rings; the vector engine runs an in-place halving
    max-tree per unit as soon as that unit's load lands.  Cells of a window
    end up in the window's first 256 columns.
    """
    from contextlib import ExitStack

    import concourse.bass as bass
    import concourse.mybir as mybir

    nc = bass.Bass()
    f16 = mybir.dt.float16
    x = nc.declare_dram_parameter("x", [TGT, SRC], f16, isOutput=False)
    bm = nc.declare_dram_parameter("bm", [P, NCHUNK, C], f16, isOutput=True)

    with ExitStack() as ctx:
        tiles = ctx.enter_context(nc.sbuf_tensor([P, NTILES, SRC], f16))
        bmsb = ctx.enter_context(nc.sbuf_tensor([P, NCHUNK, C], f16))
        s_u = [ctx.enter_context(nc.semaphore(f"s_u{j}")) for j in range(len(UNITS))]
        s_da = ctx.enter_context(nc.semaphore("s_da"))
        s_db = ctx.enter_context(nc.semaphore("s_db"))
        s_dc = ctx.enter_context(nc.semaphore("s_dc"))
        s_out = ctx.enter_context(nc.semaphore("s_out"))
        block = ctx.enter_context(nc.Block())

        # DRAM view [p, tile, col] of the row-major [2048, 4096] head slice
        xv = x[:, :].rearrange("(t p) c -> p t c", p=P)

        def unit_src(slot0, nslots):
            """(tile0, ntiles, col0, width) covered by a unit."""
            t0, c0, w = CHUNKS[slot0]
            if nslots == 2:
                t1, c1, w1 = CHUNKS[slot0 + 1]
                assert c0 == c1 == 0 and w == w1 == SRC and t1 == t0 + 1
                return t0, 2, 0, SRC
            return t0, 1, c0, w

        def issue_loads(eng, parity):
            for j, (slot0, nslots) in enumerate(UNITS):
                if j % 2 != parity:
                    continue
                t0, nt, c0, w = unit_src(slot0, nslots)
                eng.dma_start(
                    out=tiles[:, t0 : t0 + nt, c0 : c0 + w],
                    in_=xv[:, t0 : t0 + nt, c0 : c0 + w],
                ).then_inc(s_u[j], 16)

        @block.sync
        def _(sync):
            issue_loads(sync, 0)
            sync.wait_ge(s_out, 48)

        @block.scalar
        def _(scalar):
            issue_loads(scalar, 1)

        @block.gpsimd
        def _(gpsimd):
            # bm stores ride the (otherwise idle) SWDGE queue so the early
            # flushes overlap the HWDGE load stream and tail folds, leaving
            # only a tiny final store after the last fold.
            gpsimd.wait_ge(s_da, 1)
            gpsimd.dma_start(
                out=bm[:, :N_STORE_A, :], in_=bmsb[:, :N_STORE_A, :]
            ).then_inc(s_out, 16)
            gpsimd.wait_ge(s_db, 1)
            gpsimd.dma_start(
                out=bm[:, N_STORE_A:N_STORE_B, :],
                in_=bmsb[:, N_STORE_A:N_STORE_B, :],
            ).then_inc(s_out, 16)
            gpsimd.wait_ge(s_dc, 1)
            gpsimd.dma_start(
                out=bm[:, N_STORE_B:, :], in_=bmsb[:, N_STORE_B:, :]
            ).then_inc(s_out, 16)

        @block.vector
        def _(vector):
            mx = mybir.AluOpType.max

            for j, (slot0, nslots) in enumerate(UNITS):
                vector.wait_ge(s_u[j], 16)
                t0, nt, c0, w0 = unit_src(slot0, nslots)
                w = w0 // 2
                while w >= C:
                    nc.vector.tensor_tensor(
                        out=tiles[:, t0 : t0 + nt, c0 : c0 + w],
                        in0=tiles[:, t0 : t0 + nt, c0 : c0 + w],
                        in1=tiles[:, t0 : t0 + nt, c0 + w : c0 + 2 * w],
                        op=mx,
                    )
                    w //= 2
                if t0 + nt == 10 and c0 == 0 and nt == 2:
                    # tiles 0..9 folded: flush chunk slots 0..11.
                    # slots 0-3 = tile 0/1 half windows, slots 4-11 = t2..t9.
                    nc.vector.tensor_copy(
                        out=bmsb[:, 0:4, :].rearrange(
                            "p (t w) c -> p t w c", t=2
                        ),
                        in_=tiles[:, 0:2, :].rearrange(
                            "p t (w c) -> p t w c", w=2
                        )[:, :, :, 0:C],
                    )
                    nc.vector.tensor_copy(
                        out=bmsb[:, 4:N_STORE_A, :], in_=tiles[:, 2:10, 0:C]
                    ).then_inc(s_da, 1)
                if t0 == 14 and nt == 1 and c0 == 0:
                    # t14 folded: flush slots 12..16 (t10..t14)
                    nc.vector.tensor_copy(
                        out=bmsb[:, N_STORE_A:N_STORE_B, :],
                        in_=tiles[:, 10:15, 0:C],
                    ).then_inc(s_db, 1)
            # tile-15 cells: slot 17 = window [0,2048), slot 18 = [2048,3072),
            # slot 19 = [3072,4096).
            nc.vector.tensor_copy(
                out=bmsb[:, N_STORE_B, :], in_=tiles[:, 15, 0:C]
            )
            nc.vector.tensor_copy(
                out=bmsb[:, N_STORE_B + 1 :, :].rearrange(
                    "p (t w) c -> p t w c", t=1
                ),
                in_=tiles[:, 15:16, SRC // 2 :].rearrange(
                    "p t (w c) -> p t w c", w=2
                )[:, :, :, 0:C],
            ).then_inc(s_dc, 1)

    return nc


def _get_nc():
    if "nc" not in _cache:
        _cache["nc"] = _build_nc()
    return _cache["nc"]


def run_device(aw16, **run_kwargs):
    """Run the per-head cell-max kernel on 8 cores.

    aw16: [H, TGT, SRC] float16. Returns ([H, P, NCHUNK, C] float16, results).
    """
    from concourse.bass_utils import run_bass_kernel_spmd

    nc = _get_nc()
    in_maps = [{"x": np.ascontiguousarray(aw16[c])} for c in range(H)]
    res = run_bass_kernel_spmd(nc, in_maps, list(range(H)), **run_kwargs)
    bm = np.stack([res.results[c]["bm"] for c in range(H)])
    return bm, res


def _host_cellmax(aw16):
    """Numpy fallback producing the same [H, P, NCHUNK, C] cell maxima."""
    bm = np.empty((H, P, NCHUNK, C), np.float16)
    for s, (t, c0, w) in enumerate(CHUNKS):
        rows = aw16[:, t * P : (t + 1) * P, c0 : c0 + w]
        bm[:, :, s, :] = rows.reshape(H, P, w // C, C).max(axis=2)
    return bm


def _exact_argmax(aw, bm):
    """Exact first-occurrence np.argmax(aw, -1) from device cell maxima.

    aw: [H, TGT, SRC] float32; bm: [H, P, NCHUNK, C] float16 with the CHUNKS
    layout: chunk slot s covers rows t*128+p of column window [c0, c0+w),
    cell c holding positions {c0 + c + 256k : k < w//256}.
    """
    BIG = 1 << 20
    cand_m, cand_pos, cand_row = [], [], []

    # per-row fp16 max over all cells of that row's chunks
    rowmax = np.full((H, TGT), -np.inf, np.float16)
    for s, (t, c0, w) in enumerate(CHUNKS):
        np.maximum(
            rowmax[:, t * P : (t + 1) * P],
            bm[:, :, s, :].max(-1),
            out=rowmax[:, t * P : (t + 1) * P],
        )

    for s, (t, c0, w) in enumerate(CHUNKS):
        depth = w // C
        mask = bm[:, :, s, :] == rowmax[:, t * P : (t + 1) * P, None]
        hs, ps, cs = np.nonzero(mask)
        if not hs.size:
            continue
        win = aw[:, t * P : (t + 1) * P, c0 : c0 + w].reshape(H, P, depth, C)
        vals = win[hs, ps, :, cs]  # [N, depth]
        m = vals.max(1)
        k = np.where(vals == m[:, None], np.arange(depth), BIG).min(1)
        cand_m.append(m)
        cand_pos.append(c0 + cs + C * k)
        cand_row.append(hs * TGT + (t * P + ps))

    m = np.concatenate(cand_m)
    pos = np.concatenate(cand_pos)
    row = np.concatenate(cand_row)
    order = np.argsort(row, kind="stable")
    m, pos, row = m[order], pos[order], row[order]
    starts = np.flatnonzero(np.r_[True, row[1:] != row[:-1]])
    urow = row[starts]
    assert urow.size == H * TGT, f"missing rows: {urow.size}"
    best = np.maximum.reduceat(m, starts)
    seg = np.repeat(np.arange(starts.size), np.diff(np.r_[starts, row.size]))
    bestpos = np.minimum.reduceat(
        np.where(m == best[seg], pos, 1 << 30), starts
    )
    out = np.empty(H * TGT, np.int64)
    out[urow] = bestpos
    return out.reshape(H, TGT)


def kernel(attention_weight):
    aw = np.asarray(attention_weight)
    assert aw.shape == (H, TGT, SRC), aw.shape
    aw = aw.astype(np.float32, copy=False)
    aw16 = aw.astype(np.float16)

    try:
        bm, _ = run_device(aw16)
    except Exception as e:  # device path failed: fall back to host cellmax
        import traceback

        traceback.print_exc()
        print(f"WARNING: device path failed ({e!r}); falling back to numpy")
        bm = _host_cellmax(aw16)

    cand = _exact_argmax(aw, bm)  # [H, TGT]
    present = np.zeros((H, SRC), np.float32)
    present[np.arange(H)[:, None], cand] = 1.0
    counting = present.sum(axis=0)

    if counting.max() <= K_THRESH:
        return np.broadcast_to(aw[SAMPLED_T0], aw.shape).copy()
    return aw
